# revision 44
# baseline (speedup 1.0000x reference)
"""ViT-style attention with decomposed relative position embeddings on 8 TRN2
NeuronCores. Data-parallel over batch (B=8 -> 1 image per core); weights and
the small rel-pos tables are replicated.

Schedule: head-group pipelined so the ScalarE exp stream starts early and
the PE never starves:
  - group A = heads 0-5, group B = heads 6-11
  - qk o-tiles A (q side) -> rel fold A with k o-tiles interleaved ->
    attention A
  - group B work (qk o-tiles, v GEMM halves, rel fold B) and projection
    passes 1-2 are emitted as PE fill items inside the attention S loops, so
    the PE stays dense (keeps the p-state at 2.4 GHz) while ACT chews exp
  - projection is 3-pass: chunks 0-2 (heads 0-5) fill heads 7-9, chunks 3-4
    fill heads 10-11 (partials parked in SBUF as bf16), chunk 5 + final add
    after the last normalization
One PSUM pool with three tags lives the whole kernel (no pool-boundary
barriers): "big" [128,1024]f32 x2 (qk o-tiles / S / proj pass 3), "pv"
[65,512]f32 x2 (PV hf halves), "fill" [128,512]f32 x2 (v, rel accr, group-B
o-tile halves, proj passes 1-2).

qext layout is qx-major [128, a, h, b] so rel-fold evacuations are (nearly)
contiguous copies instead of strided scatters. The rel-pos additions ride in
the same 128-deep contraction as q.k (rows 64:128 = rel_h/rel_w folds vs
onehot key rows). The appended ones column of Vaug makes PSUM row 64 the
softmax denominator; it is reshaped [1,1024] -> [32,32] by DMA so the DVE
reciprocal runs on 32 lanes, then gpsimd partition-broadcast + DVE multiply.

All matmuls run in bf16 (fp32 PSUM accumulation).
"""

import numpy as np
import ml_dtypes

BF16 = ml_dtypes.bfloat16

B, H, W, C = 8, 32, 32, 768
NH, HD, T = 12, 64, 1024
N_CORES = 8

_cache = {}


def _bf(a):
    return np.ascontiguousarray(np.asarray(a, dtype=np.float32)).astype(BF16)


def _f32(a):
    return np.ascontiguousarray(np.asarray(a, dtype=np.float32))


def _build_nc():
    if "nc" in _cache:
        return _cache["nc"]

    import concourse.mybir as mybir
    import concourse.tile as tile
    from concourse import bacc

    f32 = mybir.dt.float32
    bf16 = mybir.dt.bfloat16
    EXP = mybir.ActivationFunctionType.Exp

    nc = bacc.Bacc("TRN2", target_bir_lowering=False, debug=False)

    # ---- DRAM I/O ----
    xT_d = nc.dram_tensor("xT", [C, T], bf16, kind="ExternalInput")
    wqk_d = nc.dram_tensor("w_qk", [C, 2 * C], bf16, kind="ExternalInput")
    wv_d = nc.dram_tensor("w_v", [C, C], bf16, kind="ExternalInput")
    wp_d = nc.dram_tensor("w_p", [C, C], bf16, kind="ExternalInput")
    bqk_d = nc.dram_tensor("b_qk", [128, 12], f32, kind="ExternalInput")
    bv_d = nc.dram_tensor("b_v", [1, C], f32, kind="ExternalInput")
    bp_d = nc.dram_tensor("b_p", [1, C], f32, kind="ExternalInput")
    relt_d = nc.dram_tensor("relt", [64, 2048], bf16, kind="ExternalInput")
    oneh_d = nc.dram_tensor("onehot", [64, T], bf16, kind="ExternalInput")
    iden_d = nc.dram_tensor("ident", [128, 128], bf16, kind="ExternalInput")
    ones_d = nc.dram_tensor("ones64", [1, 64], bf16, kind="ExternalInput")
    out_d = nc.dram_tensor("out", [T, C], f32, kind="ExternalOutput")

    with tile.TileContext(nc) as tc:
        with tc.tile_pool(name="const", bufs=1) as cp, \
             tc.tile_pool(name="ps", bufs=2, space="PSUM") as ps, \
             tc.tile_pool(name="sb", bufs=2) as sb:
            # ---- persistent SBUF tensors ----
            xT = cp.tile([128, 6, T], bf16, tag="xT")
            wqk = cp.tile([128, 6, 2 * C], bf16, tag="wqk")
            wv = cp.tile([128, 6, C], bf16, tag="wv")
            wpr = cp.tile([128, 6, C], bf16, tag="wpr")
            bqk = cp.tile([128, 12], f32, tag="bqk")
            bv_row = cp.tile([1, C], f32, tag="bv_row")
            bp_row = cp.tile([1, C], f32, tag="bp_row")
            bv_bc = cp.tile([128, C], f32, tag="bv_bc")
            bp_bc = cp.tile([128, C], f32, tag="bp_bc")
            relt = cp.tile([64, 2048], bf16, tag="relt")
            oneh = cp.tile([64, T], bf16, tag="oneh")
            iden = cp.tile([128, 128], bf16, tag="iden")
            ones64 = cp.tile([1, 64], bf16, tag="ones64")
            # qx-major extended q: [p, a(row idx), h(6), b(col idx)]
            # p 0:64 = q/8 channels, 64:96 rel_h fold, 96:128 rel_w fold
            qA = cp.tile([128, 32, 6, 32], bf16, tag="qA")
            qB = cp.tile([128, 32, 6, 32], bf16, tag="qB")
            # extended k: p 0:64 = k channels, 64:128 = onehot rows
            kxA = cp.tile([128, 6, T], bf16, tag="kxA")
            kxB = cp.tile([128, 6, T], bf16, tag="kxB")
            # v augmented with ones column (softmax denominator)
            vgA = cp.tile([128, 8, 6, 65], bf16, tag="vgA")
            vgB = cp.tile([128, 8, 6, 65], bf16, tag="vgB")
            yall = cp.tile([128, 6, T], bf16, tag="yall")
            zpart = cp.tile([128, 8, C], bf16, tag="zpart")

            # ---- input DMAs, critical-path first ----
            # group-A qk weight columns (q heads 0-5 cols 0:384, k heads 0-5
            # cols 768:1152) land interleaved with xT so the first o-tiles
            # unblock ~5us earlier than a monolithic wqk transfer
            nc.sync.dma_start(bqk[:], bqk_d[:])
            for c in range(6):
                nc.sync.dma_start(xT[:, c, :], xT_d[c * 128:(c + 1) * 128, :])
                nc.sync.dma_start(wqk[:, c, 0:384],
                                  wqk_d[c * 128:(c + 1) * 128, 0:384])
                nc.sync.dma_start(wqk[:, c, 768:1152],
                                  wqk_d[c * 128:(c + 1) * 128, 768:1152])
            nc.sync.dma_start(relt[:], relt_d[:])
            nc.sync.dma_start(oneh[:], oneh_d[:])
            nc.sync.dma_start(iden[:], iden_d[:])
            nc.sync.dma_start(ones64[:], ones_d[:])
            for c in range(6):
                nc.sync.dma_start(wqk[:, c, 384:768],
                                  wqk_d[c * 128:(c + 1) * 128, 384:768])
                nc.sync.dma_start(wqk[:, c, 1152:1536],
                                  wqk_d[c * 128:(c + 1) * 128, 1152:1536])
            for c in range(6):
                nc.sync.dma_start(wv[:, c, :], wv_d[c * 128:(c + 1) * 128, :])
            nc.sync.dma_start(bv_row[:], bv_d[:])
            nc.sync.dma_start(bp_row[:], bp_d[:])
            for c in range(6):
                nc.sync.dma_start(wpr[:, c, :], wp_d[c * 128:(c + 1) * 128, :])
            nc.gpsimd.partition_broadcast(bv_bc[:], bv_row[:])
            nc.gpsimd.partition_broadcast(bp_bc[:], bp_row[:])
            nc.gpsimd.memset(vgA[:, :, :, 64:65], 1.0)
            nc.gpsimd.memset(vgB[:, :, :, 64:65], 1.0)

            def onehot_copy(j):
                """Replicate onehot into kext rows 64:128 (j in 0..11).
                Emitted inside the rel-A loop so these copies don't head the
                DVE queue in front of the o-tile evacuations."""
                kx, hl = (kxA, j) if j < 6 else (kxB, j - 6)
                if j % 3 == 0:
                    nc.gpsimd.tensor_copy(kx[64:128, hl, :], oneh[:])
                elif j % 3 == 1:
                    nc.vector.tensor_copy(kx[64:128, hl, :], oneh[:])
                else:
                    nc.scalar.copy(kx[64:128, hl, :], oneh[:])

            # ================= building blocks =================
            def qk_otile(ot):
                """Full-width qk o-tile (pre-attention, tag 'big')."""
                acc = ps.tile([128, T], f32, tag="big")
                for c in range(6):
                    for hf in range(2):
                        nc.tensor.matmul(
                            acc[:, hf * 512:(hf + 1) * 512],
                            wqk[:, c, ot * 128:(ot + 1) * 128],
                            xT[:, c, hf * 512:(hf + 1) * 512],
                            start=(c == 0), stop=(c == 5),
                        )
                _qk_evac(acc, ot, 0, T, split_eng=True)

            def qk_otile_half(ot, hf):
                """Half-width qk o-tile (attention fill, tag 'fill')."""
                acc = ps.tile([128, 512], f32, tag="fill")
                for c in range(6):
                    nc.tensor.matmul(
                        acc[:],
                        wqk[:, c, ot * 128:(ot + 1) * 128],
                        xT[:, c, hf * 512:(hf + 1) * 512],
                        start=(c == 0), stop=(c == 5),
                    )
                _qk_evac(acc, ot, hf * 512, 512, split_eng=False)

            def _qk_evac(acc, ot, col0, ncols, split_eng):
                is_q = ot < 6
                hp = ot if is_q else ot - 6
                qx = qA if hp < 3 else qB
                kx = kxA if hp < 3 else kxB
                for half in range(2):
                    head = 2 * hp + half
                    hl = head % 6
                    src = acc[64 * half:64 * (half + 1), 0:ncols]
                    bias = bqk[64 * half:64 * (half + 1), ot:ot + 1]
                    if is_q:
                        a0, a1 = col0 // 32, (col0 + ncols) // 32
                        dst = qx[0:64, a0:a1, hl, :]
                        src = src.rearrange("p (a b) -> p a b", b=32)
                    else:
                        dst = kx[0:64, hl, col0:col0 + ncols]
                    if split_eng and (half == 1 or ot >= 7):
                        # ACT is idle pre-attention; DVE is the longer queue
                        nc.scalar.add(dst, src, bias)
                    else:
                        nc.vector.tensor_scalar_add(dst, src, bias)

            def rel_iter(grp, i, eng):
                """Fold rel tables for qx pair (2i, 2i+1) into q{A,B} rows
                64:128. eng picks the evacuation engine pair. Pre-attention
                (grp 0) borrows the idle 'pv' ring so the 'fill' ring stays
                free for v tiles."""
                qx_t = qA if grp == 0 else qB
                accr = ps.tile([128, 2, 192], f32,
                               tag=("pv" if grp == 0 else "fill"))
                for g in range(2):
                    qx = 2 * i + g
                    for tbl in range(2):
                        m = 2 + tbl
                        lhsT = relt[0:64, tbl * 1024 + qx * 32:
                                    tbl * 1024 + qx * 32 + 32]
                        if tbl == 0:
                            rhs = qx_t[0:64, qx, :, :]      # [64, 6, 32]
                        else:
                            rhs = qx_t[0:64, :, :, qx]      # [64, 32, 6]
                        nc.tensor.matmul(
                            accr[32 * m:32 * (m + 1), g, :],
                            lhsT, rhs,
                            start=True, stop=True,
                            tile_position=(0, 32 * m),
                        )
                dst_h = qx_t[64:96, 2 * i:2 * i + 2, :, :]
                src_h = accr[64:96, :, :].rearrange("p g (h b) -> p g h b", h=6)
                dst_w = qx_t[96:128, :, :, 2 * i:2 * i + 2]
                src_w = accr[96:128, :, :].rearrange(
                    "p g (a h) -> p a h g", a=32)
                if eng == 0:        # pre-attention: ACT is idle
                    nc.scalar.copy(dst_h, src_h)
                    nc.vector.tensor_copy(dst_w, src_w)
                elif eng == 1:
                    nc.vector.tensor_copy(dst_h, src_h)
                    nc.scalar.copy(dst_w, src_w)
                else:               # in attention: keep ACT strictly for
                    # exp -- any other op on the in-order ACT queue blocks
                    # the exp stream on its own (possibly slow) producers
                    nc.vector.tensor_copy(dst_h, src_h)
                    nc.vector.tensor_copy(dst_w, src_w)

            def v_half(tt, grp):
                """v GEMM for token tile tt, head group grp (6 heads)."""
                vg = vgA if grp == 0 else vgB
                accv = ps.tile([128, 384], f32, tag="fill")
                for c in range(6):
                    nc.tensor.matmul(
                        accv[:],
                        xT[:, c, tt * 128:(tt + 1) * 128],
                        wv[:, c, grp * 384:(grp + 1) * 384],
                        start=(c == 0), stop=(c == 5),
                    )
                nc.vector.tensor_add(
                    vg[:, tt, :, 0:64],
                    accv[:].rearrange("p (h d) -> p h d", h=6),
                    bv_bc[:, grp * 384:(grp + 1) * 384].rearrange(
                        "p (h d) -> p h d", h=6))

            def proj_pass(tt, half, p0, p1, first):
                """Partial projection over weight chunks [p0, p1) for token
                tile tt, output half (0: cols 0:512, 1: cols 512:768).
                Accumulates into zpart (bf16, bias folded in on the first
                pass)."""
                ncols = 512 if half == 0 else 256
                c0 = half * 512
                accz = ps.tile([128, ncols], f32, tag="fill", name="accz")
                for p in range(p0, p1):
                    nc.tensor.matmul(
                        accz[:],
                        yall[:, p, tt * 128:(tt + 1) * 128],
                        wpr[:, p, c0:c0 + ncols],
                        start=(p == p0), stop=(p == p1 - 1),
                    )
                dst = zpart[:, tt, c0:c0 + ncols]
                if first:
                    nc.vector.tensor_add(dst, accz[:], bp_bc[:, c0:c0 + ncols])
                else:
                    nc.vector.tensor_add(dst, dst, accz[:])

            def head_attn(h, fills):
                """S -> exp -> PV -> normalize for head h, emitting items from
                `fills` (list of thunks) between PE bursts. All fills land in
                the S loop so anything a later PV consumes is already in PE
                program order."""
                grp = 0 if h < 6 else 1
                hl = h % 6
                qx_t, kx, vg = (qA, kxA, vgA) if grp == 0 else (qB, kxB, vgB)
                p_t = sb.tile([128, 8, T], bf16, tag="P")
                nf = len(fills)
                counts = [nf // 4 + (1 if j < nf % 4 else 0) for j in range(4)]
                fi = 0
                for kt in range(8):
                    accs = ps.tile([128, T], f32, tag="big")
                    for hf in range(2):
                        nc.tensor.matmul(
                            accs[:, hf * 512:(hf + 1) * 512],
                            kx[:, hl, kt * 128:(kt + 1) * 128],
                            qx_t[:, hf * 16:(hf + 1) * 16, hl, :],
                            start=True, stop=True,
                        )
                    nc.scalar.activation(p_t[:, kt, :], accs[:], EXP)
                    if kt % 2 == 1:
                        for _ in range(counts[kt // 2]):
                            fills[fi]()
                            fi += 1
                # PV hf-outer: half 0 finishes ~1.7us before half 1, so its
                # normalization chain (and the psum slot release) overlaps
                # the second half's accumulation.
                drow = sb.tile([1, T], f32, tag="drow", bufs=1)
                dsq = sb.tile([32, 2, 16], f32, tag="dsq", bufs=1)
                pvsb = sb.tile([64, 2, 512], bf16, tag="pvsb")
                rbc = sb.tile([64, 2, 512], f32, tag="rbc")
                rh = slice(64 * (h % 2), 64 * (h % 2) + 64)
                for hf in range(2):
                    accp = ps.tile([65, 512], f32, tag="pv", name=f"pv{hf}")
                    for kt in range(8):
                        nc.tensor.matmul(
                            accp[:],
                            vg[:, kt, hl, :],
                            p_t[:, kt, hf * 512:(hf + 1) * 512],
                            start=(kt == 0), stop=(kt == 7),
                        )
                    cols = slice(hf * 512, (hf + 1) * 512)
                    nc.vector.tensor_copy(drow[:, cols], accp[64:65, :])
                    nc.vector.tensor_copy(pvsb[:, hf, :], accp[0:64, :])
                    nc.sync.dma_start(dsq[:, hf, :], drow[:, cols])
                    if h == 11:
                        # last head: PE is idle here and gpsimd queue latency
                        # gates the projection tail -- broadcast the
                        # reciprocal row via a 1-deep ones matmul instead
                        dsqb = sb.tile([32, 2, 16], bf16, tag="dsqb", bufs=1)
                        drob = sb.tile([1, T], bf16, tag="drob", bufs=1)
                        with nc.allow_low_precision(
                                reason="bf16 recip row for last-head bcast"):
                            nc.vector.reciprocal(dsqb[:, hf, :],
                                                 dsq[:, hf, :])
                        nc.sync.dma_start(drob[:, cols], dsqb[:, hf, :])
                        rbcp = ps.tile([64, 512], f32, tag="fill", name="rbcp")
                        nc.tensor.matmul(rbcp[:], ones64[:],
                                         drob[:, cols], start=True, stop=True)
                        nc.vector.tensor_mul(
                            yall[rh, h // 2, cols], pvsb[:, hf, :], rbcp[:])
                    else:
                        nc.vector.reciprocal(dsq[:, hf, :], dsq[:, hf, :])
                        nc.sync.dma_start(drow[:, cols], dsq[:, hf, :])
                        nc.gpsimd.partition_broadcast(rbc[:, hf, :],
                                                      drow[:, cols])
                        nc.vector.tensor_mul(
                            yall[rh, h // 2, cols],
                            pvsb[:, hf, :],
                            rbc[:, hf, :])

            # ================= phase A: group-A qk + rel =================
            for ot in (0, 1, 2):
                qk_otile(ot)
            # k o-tiles interleaved into the rel-A loop: PE stays busy while
            # ACT/DVE drain the rel psum tiles
            k_sched = {1: 6, 6: 7, 11: 8}
            for i in range(16):
                if i in k_sched:
                    qk_otile(k_sched[i])
                rel_iter(0, i, i % 2)
                if i < 12:
                    onehot_copy(i)
                else:
                    v_half(i - 12, 0)

            # ============ attention with fill items ============
            # remaining group-A v halves go right before attention so head
            # 0's S loop feeds the exp stream without PE detours
            for tt in range(4, 8):
                v_half(tt, 0)

            fills_by_head = {
                1: [lambda o=o, f=f: qk_otile_half(o, f)
                    for o, f in ((3, 0), (3, 1), (4, 0), (4, 1))],
                2: [lambda o=o, f=f: qk_otile_half(o, f)
                    for o, f in ((5, 0), (5, 1))]
                   + [lambda i=i: rel_iter(1, i, 2) for i in range(2)],
                3: [lambda i=i: rel_iter(1, i, 2) for i in range(2, 8)]
                   + [lambda tt=tt: v_half(tt, 1) for tt in range(2)],
                4: [lambda i=i: rel_iter(1, i, 2) for i in range(8, 14)]
                   + [lambda o=o, f=f: qk_otile_half(o, f)
                      for o, f in ((9, 0), (9, 1))]
                   + [lambda tt=tt: v_half(tt, 1) for tt in range(2, 4)],
                5: [lambda i=i: rel_iter(1, i, 2) for i in range(14, 16)]
                   + [lambda o=o, f=f: qk_otile_half(o, f)
                      for o, f in ((10, 0), (10, 1))]
                   + [lambda tt=tt: v_half(tt, 1) for tt in range(4, 6)],
                6: [lambda o=o, f=f: qk_otile_half(o, f)
                    for o, f in ((11, 0), (11, 1))]
                   + [lambda tt=tt: v_half(tt, 1) for tt in range(6, 8)],
                7: [lambda tt=tt, hf=hf: proj_pass(tt, hf, 0, 3, True)
                    for tt in range(3) for hf in range(2)],
                8: [lambda tt=tt, hf=hf: proj_pass(tt, hf, 0, 3, True)
                    for tt in range(3, 6) for hf in range(2)],
                9: [lambda tt=tt, hf=hf: proj_pass(tt, hf, 0, 3, True)
                    for tt in range(6, 8) for hf in range(2)],
                10: [lambda tt=tt, hf=hf: proj_pass(tt, hf, 3, 5, False)
                     for tt in range(6) for hf in range(2)],
                11: [lambda tt=tt, hf=hf: proj_pass(tt, hf, 3, 5, False)
                     for tt in range(6, 8) for hf in range(2)],
            }
            for h in range(12):
                head_attn(h, fills_by_head.get(h, []))

            # ============ projection pass 3 (chunk 5) + final add ============
            # the zpart partial rides into PSUM via an identity matmul and the
            # evacuation runs on ACT (idle once the exp stream drains), so the
            # tail isn't serialized on DVE adds
            for tt in range(8):
                accz = ps.tile([128, C], f32, tag="big")
                for c0, nc_ in ((0, 512), (512, 256)):
                    nc.tensor.matmul(
                        accz[:, c0:c0 + nc_],
                        yall[:, 5, tt * 128:(tt + 1) * 128],
                        wpr[:, 5, c0:c0 + nc_],
                        start=True, stop=False,
                    )
                    nc.tensor.matmul(
                        accz[:, c0:c0 + nc_],
                        iden[:],
                        zpart[:, tt, c0:c0 + nc_],
                        start=False, stop=True,
                    )
                z_t = sb.tile([128, C], f32, tag="Zt")
                nc.scalar.copy(z_t[:], accz[:])
                nc.sync.dma_start(out_d[tt * 128:(tt + 1) * 128, :], z_t[:])

    nc.compile()
    _cache["nc"] = nc
    return nc


def _host_prep(x, w_qkv, b_qkv, w_proj, b_proj, rel_pos_h, rel_pos_w):
    scale = HD ** -0.5
    w_qkv = _f32(w_qkv)
    b_qkv = _f32(b_qkv)

    w_qk = w_qkv[:, : 2 * C].copy()
    w_qk[:, :C] *= scale
    b_qk_flat = b_qkv[: 2 * C].copy()
    b_qk_flat[:C] *= scale
    b_qk = np.ascontiguousarray(b_qk_flat.reshape(12, 128).T)  # [128, 12]

    # relt [64, 2048]: cols tbl*1024 + qx*32 + j -> 8*rel_pos[qx - j + 31, :]
    idx = np.arange(32)[:, None] - np.arange(32)[None, :] + 31  # [qx, j]
    relt = np.concatenate(
        [
            (8.0 * _f32(rel_pos_h))[idx].transpose(2, 0, 1).reshape(64, 1024),
            (8.0 * _f32(rel_pos_w))[idx].transpose(2, 0, 1).reshape(64, 1024),
        ],
        axis=1,
    )

    k = np.arange(T)
    onehot = np.zeros((64, T), np.float32)
    onehot[k // 32, k] = 1.0
    onehot[32 + k % 32, k] = 1.0

    shared = {
        "w_qk": _bf(w_qk),
        "w_v": _bf(w_qkv[:, 2 * C:]),
        "w_p": _bf(w_proj),
        "b_qk": _f32(b_qk),
        "b_v": _f32(b_qkv[2 * C:])[None, :],
        "b_p": _f32(b_proj)[None, :],
        "relt": _bf(relt),
        "onehot": _bf(onehot),
        "ident": _bf(np.eye(128, dtype=np.float32)),
        "ones64": _bf(np.ones((1, 64), dtype=np.float32)),
    }
    x = _f32(x)
    in_maps = []
    for i in range(N_CORES):
        m = dict(shared)
        m["xT"] = _bf(x[i].reshape(T, C).T)
        in_maps.append(m)
    return in_maps


def kernel(x, w_qkv, b_qkv, w_proj, b_proj, rel_pos_h, rel_pos_w):
    from concourse.bass_utils import run_bass_kernel_spmd

    nc = _build_nc()
    in_maps = _host_prep(x, w_qkv, b_qkv, w_proj, b_proj, rel_pos_h, rel_pos_w)
    res = run_bass_kernel_spmd(nc, in_maps, core_ids=list(range(N_CORES)))
    out = np.stack([_f32(res.results[i]["out"]) for i in range(N_CORES)])
    return out.reshape(B, H, W, C)


# revision 45
# speedup vs baseline: 1.2163x; 1.2163x over previous
"""ViT-style attention with decomposed relative position embeddings on 8 TRN2
NeuronCores. Data-parallel over batch (B=8 -> 1 image per core); weights and
the small rel-pos tables are replicated.

Schedule: head-group pipelined so the ScalarE exp stream starts early and
the PE never starves:
  - group A = heads 0-5, group B = heads 6-11
  - qk o-tiles A (q side) -> rel fold A with k o-tiles interleaved ->
    attention A
  - group B work (qk o-tiles, v GEMM halves, rel fold B) and projection
    passes 1-2 are emitted as PE fill items inside the attention S loops, so
    the PE stays dense (keeps the p-state at 2.4 GHz) while ACT chews exp
  - projection is 3-pass: chunks 0-2 (heads 0-5) fill heads 7-9, chunks 3-4
    fill heads 10-11 (partials parked in SBUF as bf16), chunk 5 + final add
    after the last normalization
One PSUM pool with three tags lives the whole kernel (no pool-boundary
barriers): "big" [128,1024]f32 x2 (qk o-tiles / S / proj pass 3), "pv"
[65,512]f32 x2 (PV hf halves), "fill" [128,512]f32 x2 (v, rel accr, group-B
o-tile halves, proj passes 1-2).

qext layout is qx-major [128, a, h, b] so rel-fold evacuations are (nearly)
contiguous copies instead of strided scatters. The rel-pos additions ride in
the same 128-deep contraction as q.k (rows 64:128 = rel_h/rel_w folds vs
onehot key rows). The appended ones column of Vaug makes PSUM row 64 the
softmax denominator; it is reshaped [1,1024] -> [32,32] by DMA so the DVE
reciprocal runs on 32 lanes, then gpsimd partition-broadcast + DVE multiply.

All matmuls run in bf16 (fp32 PSUM accumulation).
"""

import numpy as np
import ml_dtypes

BF16 = ml_dtypes.bfloat16

B, H, W, C = 8, 32, 32, 768
NH, HD, T = 12, 64, 1024
N_CORES = 8

_cache = {}


def _bf(a):
    return np.ascontiguousarray(np.asarray(a, dtype=np.float32)).astype(BF16)


def _f32(a):
    return np.ascontiguousarray(np.asarray(a, dtype=np.float32))


def _build_nc():
    if "nc" in _cache:
        return _cache["nc"]

    import concourse.mybir as mybir
    import concourse.tile as tile
    from concourse import bacc

    f32 = mybir.dt.float32
    bf16 = mybir.dt.bfloat16
    EXP = mybir.ActivationFunctionType.Exp

    nc = bacc.Bacc("TRN2", target_bir_lowering=False, debug=False)

    # ---- DRAM I/O ----
    xT_d = nc.dram_tensor("xT", [C, T], bf16, kind="ExternalInput")
    wqk_d = nc.dram_tensor("w_qk", [C, 2 * C], bf16, kind="ExternalInput")
    wv_d = nc.dram_tensor("w_v", [C, C], bf16, kind="ExternalInput")
    wp_d = nc.dram_tensor("w_p", [C, C], bf16, kind="ExternalInput")
    bqk_d = nc.dram_tensor("b_qk", [128, 12], f32, kind="ExternalInput")
    bv_d = nc.dram_tensor("b_v", [1, C], f32, kind="ExternalInput")
    bp_d = nc.dram_tensor("b_p", [1, C], f32, kind="ExternalInput")
    relt_d = nc.dram_tensor("relt", [64, 2048], bf16, kind="ExternalInput")
    oneh_d = nc.dram_tensor("onehot", [64, T], bf16, kind="ExternalInput")
    iden_d = nc.dram_tensor("ident", [128, 128], bf16, kind="ExternalInput")
    ones_d = nc.dram_tensor("ones64", [1, 64], bf16, kind="ExternalInput")
    out_d = nc.dram_tensor("out", [T, C], f32, kind="ExternalOutput")

    with tile.TileContext(nc) as tc:
        with tc.tile_pool(name="const", bufs=1) as cp, \
             tc.tile_pool(name="ps", bufs=2, space="PSUM") as ps, \
             tc.tile_pool(name="sb", bufs=2) as sb:
            # ---- persistent SBUF tensors ----
            xT = cp.tile([128, 6, T], bf16, tag="xT")
            wqk = cp.tile([128, 6, 2 * C], bf16, tag="wqk")
            wv = cp.tile([128, 6, C], bf16, tag="wv")
            wpr = cp.tile([128, 6, C], bf16, tag="wpr")
            bqk = cp.tile([128, 12], f32, tag="bqk")
            bv_row = cp.tile([1, C], f32, tag="bv_row")
            bp_row = cp.tile([1, C], f32, tag="bp_row")
            bv_bc = cp.tile([128, C], f32, tag="bv_bc")
            bp_bc = cp.tile([128, C], f32, tag="bp_bc")
            relt = cp.tile([64, 2048], bf16, tag="relt")
            oneh = cp.tile([64, T], bf16, tag="oneh")
            iden = cp.tile([128, 128], bf16, tag="iden")
            ones64 = cp.tile([1, 64], bf16, tag="ones64")
            # qx-major extended q: [p, a(row idx), h(6), b(col idx)]
            # p 0:64 = q/8 channels, 64:96 rel_h fold, 96:128 rel_w fold
            qA = cp.tile([128, 32, 6, 32], bf16, tag="qA")
            qB = cp.tile([128, 32, 6, 32], bf16, tag="qB")
            # extended k: p 0:64 = k channels, 64:128 = onehot rows
            kxA = cp.tile([128, 6, T], bf16, tag="kxA")
            kxB = cp.tile([128, 6, T], bf16, tag="kxB")
            # v augmented with ones column (softmax denominator)
            vgA = cp.tile([128, 8, 6, 65], bf16, tag="vgA")
            vgB = cp.tile([128, 8, 6, 65], bf16, tag="vgB")
            yall = cp.tile([128, 6, T], bf16, tag="yall")
            zpart = cp.tile([128, 8, C], bf16, tag="zpart")

            # ---- input DMAs, critical-path first ----
            # group-A qk weight columns (q heads 0-5 cols 0:384, k heads 0-5
            # cols 768:1152) land interleaved with xT so the first o-tiles
            # unblock ~5us earlier than a monolithic wqk transfer
            nc.sync.dma_start(bqk[:], bqk_d[:])
            for c in range(6):
                nc.sync.dma_start(xT[:, c, :], xT_d[c * 128:(c + 1) * 128, :])
                nc.sync.dma_start(wqk[:, c, 0:384],
                                  wqk_d[c * 128:(c + 1) * 128, 0:384])
                nc.sync.dma_start(wqk[:, c, 768:1152],
                                  wqk_d[c * 128:(c + 1) * 128, 768:1152])
            nc.sync.dma_start(relt[:], relt_d[:])
            nc.sync.dma_start(oneh[:], oneh_d[:])
            nc.sync.dma_start(iden[:], iden_d[:])
            nc.sync.dma_start(ones64[:], ones_d[:])
            for c in range(6):
                nc.sync.dma_start(wqk[:, c, 384:768],
                                  wqk_d[c * 128:(c + 1) * 128, 384:768])
                nc.sync.dma_start(wqk[:, c, 1152:1536],
                                  wqk_d[c * 128:(c + 1) * 128, 1152:1536])
            for c in range(6):
                nc.sync.dma_start(wv[:, c, :], wv_d[c * 128:(c + 1) * 128, :])
            nc.sync.dma_start(bv_row[:], bv_d[:])
            nc.sync.dma_start(bp_row[:], bp_d[:])
            for c in range(6):
                nc.sync.dma_start(wpr[:, c, :], wp_d[c * 128:(c + 1) * 128, :])
            nc.gpsimd.partition_broadcast(bv_bc[:], bv_row[:])
            nc.gpsimd.partition_broadcast(bp_bc[:], bp_row[:])
            nc.gpsimd.memset(vgA[:, :, :, 64:65], 1.0)
            nc.gpsimd.memset(vgB[:, :, :, 64:65], 1.0)

            def onehot_copy(j):
                """Replicate onehot into kext rows 64:128 (j in 0..11).
                Emitted inside the rel-A loop so these copies don't head the
                DVE queue in front of the o-tile evacuations."""
                kx, hl = (kxA, j) if j < 6 else (kxB, j - 6)
                if j % 2 == 0:
                    nc.vector.tensor_copy(kx[64:128, hl, :], oneh[:])
                else:
                    nc.scalar.copy(kx[64:128, hl, :], oneh[:])

            # ================= building blocks =================
            def qk_otile(ot):
                """Full-width qk o-tile (pre-attention, tag 'big')."""
                acc = ps.tile([128, T], f32, tag="big")
                for c in range(6):
                    for hf in range(2):
                        nc.tensor.matmul(
                            acc[:, hf * 512:(hf + 1) * 512],
                            wqk[:, c, ot * 128:(ot + 1) * 128],
                            xT[:, c, hf * 512:(hf + 1) * 512],
                            start=(c == 0), stop=(c == 5),
                        )
                _qk_evac(acc, ot, 0, T, split_eng=True)

            def qk_otile_half(ot, hf):
                """Half-width qk o-tile (attention fill, tag 'fill')."""
                acc = ps.tile([128, 512], f32, tag="fill")
                for c in range(6):
                    nc.tensor.matmul(
                        acc[:],
                        wqk[:, c, ot * 128:(ot + 1) * 128],
                        xT[:, c, hf * 512:(hf + 1) * 512],
                        start=(c == 0), stop=(c == 5),
                    )
                _qk_evac(acc, ot, hf * 512, 512, split_eng=False)

            def _qk_evac(acc, ot, col0, ncols, split_eng):
                is_q = ot < 6
                hp = ot if is_q else ot - 6
                qx = qA if hp < 3 else qB
                kx = kxA if hp < 3 else kxB
                for half in range(2):
                    head = 2 * hp + half
                    hl = head % 6
                    src = acc[64 * half:64 * (half + 1), 0:ncols]
                    bias = bqk[64 * half:64 * (half + 1), ot:ot + 1]
                    if is_q:
                        a0, a1 = col0 // 32, (col0 + ncols) // 32
                        dst = qx[0:64, a0:a1, hl, :]
                        src = src.rearrange("p (a b) -> p a b", b=32)
                    else:
                        dst = kx[0:64, hl, col0:col0 + ncols]
                    if split_eng and half == 1:   # ACT is idle pre-attention
                        nc.scalar.add(dst, src, bias)
                    else:
                        nc.vector.tensor_scalar_add(dst, src, bias)

            def rel_iter(grp, i, eng):
                """Fold rel tables for qx pair (2i, 2i+1) into q{A,B} rows
                64:128. eng picks the evacuation engine pair. Pre-attention
                (grp 0) borrows the idle 'pv' ring so the 'fill' ring stays
                free for v tiles."""
                qx_t = qA if grp == 0 else qB
                accr = ps.tile([128, 2, 192], f32,
                               tag=("pv" if grp == 0 else "fill"))
                for g in range(2):
                    qx = 2 * i + g
                    for tbl in range(2):
                        m = 2 + tbl
                        lhsT = relt[0:64, tbl * 1024 + qx * 32:
                                    tbl * 1024 + qx * 32 + 32]
                        if tbl == 0:
                            rhs = qx_t[0:64, qx, :, :]      # [64, 6, 32]
                        else:
                            rhs = qx_t[0:64, :, :, qx]      # [64, 32, 6]
                        nc.tensor.matmul(
                            accr[32 * m:32 * (m + 1), g, :],
                            lhsT, rhs,
                            start=True, stop=True,
                            tile_position=(0, 32 * m),
                        )
                dst_h = qx_t[64:96, 2 * i:2 * i + 2, :, :]
                src_h = accr[64:96, :, :].rearrange("p g (h b) -> p g h b", h=6)
                dst_w = qx_t[96:128, :, :, 2 * i:2 * i + 2]
                src_w = accr[96:128, :, :].rearrange(
                    "p g (a h) -> p a h g", a=32)
                if eng == 0:        # pre-attention: ACT is idle
                    nc.scalar.copy(dst_h, src_h)
                    nc.vector.tensor_copy(dst_w, src_w)
                elif eng == 1:
                    nc.vector.tensor_copy(dst_h, src_h)
                    nc.scalar.copy(dst_w, src_w)
                else:               # in attention: keep ACT strictly for
                    # exp -- any other op on the in-order ACT queue blocks
                    # the exp stream on its own (possibly slow) producers
                    nc.vector.tensor_copy(dst_h, src_h)
                    nc.vector.tensor_copy(dst_w, src_w)

            def v_half(tt, grp):
                """v GEMM for token tile tt, head group grp (6 heads)."""
                vg = vgA if grp == 0 else vgB
                accv = ps.tile([128, 384], f32, tag="fill")
                for c in range(6):
                    nc.tensor.matmul(
                        accv[:],
                        xT[:, c, tt * 128:(tt + 1) * 128],
                        wv[:, c, grp * 384:(grp + 1) * 384],
                        start=(c == 0), stop=(c == 5),
                    )
                nc.vector.tensor_add(
                    vg[:, tt, :, 0:64],
                    accv[:].rearrange("p (h d) -> p h d", h=6),
                    bv_bc[:, grp * 384:(grp + 1) * 384].rearrange(
                        "p (h d) -> p h d", h=6))

            def proj_pass(tt, half, p0, p1, first):
                """Partial projection over weight chunks [p0, p1) for token
                tile tt, output half (0: cols 0:512, 1: cols 512:768).
                Accumulates into zpart (bf16, bias folded in on the first
                pass)."""
                ncols = 512 if half == 0 else 256
                c0 = half * 512
                accz = ps.tile([128, ncols], f32, tag="fill", name="accz")
                for p in range(p0, p1):
                    nc.tensor.matmul(
                        accz[:],
                        yall[:, p, tt * 128:(tt + 1) * 128],
                        wpr[:, p, c0:c0 + ncols],
                        start=(p == p0), stop=(p == p1 - 1),
                    )
                dst = zpart[:, tt, c0:c0 + ncols]
                if first:
                    nc.vector.tensor_add(dst, accz[:], bp_bc[:, c0:c0 + ncols])
                else:
                    nc.vector.tensor_add(dst, dst, accz[:])

            def head_attn(h, fills):
                """S -> exp -> PV -> normalize for head h, emitting items from
                `fills` (list of thunks) between PE bursts. All fills land in
                the S loop so anything a later PV consumes is already in PE
                program order."""
                grp = 0 if h < 6 else 1
                hl = h % 6
                qx_t, kx, vg = (qA, kxA, vgA) if grp == 0 else (qB, kxB, vgB)
                p_t = sb.tile([128, 8, T], bf16, tag="P")
                nf = len(fills)
                counts = [nf // 4 + (1 if j < nf % 4 else 0) for j in range(4)]
                fi = 0
                for kt in range(8):
                    accs = ps.tile([128, T], f32, tag="big")
                    for hf in range(2):
                        nc.tensor.matmul(
                            accs[:, hf * 512:(hf + 1) * 512],
                            kx[:, hl, kt * 128:(kt + 1) * 128],
                            qx_t[:, hf * 16:(hf + 1) * 16, hl, :],
                            start=True, stop=True,
                        )
                    nc.scalar.activation(p_t[:, kt, :], accs[:], EXP)
                    if kt % 2 == 1:
                        for _ in range(counts[kt // 2]):
                            fills[fi]()
                            fi += 1
                # PV hf-outer: half 0 finishes ~1.7us before half 1, so its
                # normalization chain (and the psum slot release) overlaps
                # the second half's accumulation.
                drow = sb.tile([1, T], f32, tag="drow", bufs=1)
                dsq = sb.tile([32, 2, 16], f32, tag="dsq", bufs=1)
                pvsb = sb.tile([64, 2, 512], bf16, tag="pvsb")
                rbc = sb.tile([64, 2, 512], f32, tag="rbc")
                rh = slice(64 * (h % 2), 64 * (h % 2) + 64)
                for hf in range(2):
                    accp = ps.tile([65, 512], f32, tag="pv", name=f"pv{hf}")
                    for kt in range(8):
                        nc.tensor.matmul(
                            accp[:],
                            vg[:, kt, hl, :],
                            p_t[:, kt, hf * 512:(hf + 1) * 512],
                            start=(kt == 0), stop=(kt == 7),
                        )
                    cols = slice(hf * 512, (hf + 1) * 512)
                    nc.vector.tensor_copy(drow[:, cols], accp[64:65, :])
                    nc.vector.tensor_copy(pvsb[:, hf, :], accp[0:64, :])
                    nc.sync.dma_start(dsq[:, hf, :], drow[:, cols])
                    if h == 11:
                        # last head: PE is idle here and gpsimd queue latency
                        # gates the projection tail -- broadcast the
                        # reciprocal row via a 1-deep ones matmul instead
                        dsqb = sb.tile([32, 2, 16], bf16, tag="dsqb", bufs=1)
                        drob = sb.tile([1, T], bf16, tag="drob", bufs=1)
                        with nc.allow_low_precision(
                                reason="bf16 recip row for last-head bcast"):
                            nc.vector.reciprocal(dsqb[:, hf, :],
                                                 dsq[:, hf, :])
                        nc.sync.dma_start(drob[:, cols], dsqb[:, hf, :])
                        rbcp = ps.tile([64, 512], f32, tag="fill", name="rbcp")
                        nc.tensor.matmul(rbcp[:], ones64[:],
                                         drob[:, cols], start=True, stop=True)
                        nc.vector.tensor_mul(
                            yall[rh, h // 2, cols], pvsb[:, hf, :], rbcp[:])
                    else:
                        nc.vector.reciprocal(dsq[:, hf, :], dsq[:, hf, :])
                        nc.sync.dma_start(drow[:, cols], dsq[:, hf, :])
                        nc.gpsimd.partition_broadcast(rbc[:, hf, :],
                                                      drow[:, cols])
                        nc.vector.tensor_mul(
                            yall[rh, h // 2, cols],
                            pvsb[:, hf, :],
                            rbc[:, hf, :])

            # ================= phase A: group-A qk + rel =================
            for ot in (0, 1, 2):
                qk_otile(ot)
            # k o-tiles interleaved into the rel-A loop: PE stays busy while
            # ACT/DVE drain the rel psum tiles
            k_sched = {1: 6, 6: 7, 11: 8}
            for i in range(16):
                if i in k_sched:
                    qk_otile(k_sched[i])
                rel_iter(0, i, i % 2)
                if i < 12:
                    onehot_copy(i)
                else:
                    v_half(i - 12, 0)

            # ============ attention with fill items ============
            # remaining group-A v halves go right before attention so head
            # 0's S loop feeds the exp stream without PE detours
            for tt in range(4, 8):
                v_half(tt, 0)

            fills_by_head = {
                1: [lambda o=o, f=f: qk_otile_half(o, f)
                    for o, f in ((3, 0), (3, 1), (4, 0), (4, 1))],
                2: [lambda o=o, f=f: qk_otile_half(o, f)
                    for o, f in ((5, 0), (5, 1))]
                   + [lambda i=i: rel_iter(1, i, 2) for i in range(2)],
                3: [lambda i=i: rel_iter(1, i, 2) for i in range(2, 8)]
                   + [lambda tt=tt: v_half(tt, 1) for tt in range(2)],
                4: [lambda i=i: rel_iter(1, i, 2) for i in range(8, 14)]
                   + [lambda o=o, f=f: qk_otile_half(o, f)
                      for o, f in ((9, 0), (9, 1))]
                   + [lambda tt=tt: v_half(tt, 1) for tt in range(2, 4)],
                5: [lambda i=i: rel_iter(1, i, 2) for i in range(14, 16)]
                   + [lambda o=o, f=f: qk_otile_half(o, f)
                      for o, f in ((10, 0), (10, 1))]
                   + [lambda tt=tt: v_half(tt, 1) for tt in range(4, 6)],
                6: [lambda o=o, f=f: qk_otile_half(o, f)
                    for o, f in ((11, 0), (11, 1))]
                   + [lambda tt=tt: v_half(tt, 1) for tt in range(6, 8)],
                7: [lambda tt=tt, hf=hf: proj_pass(tt, hf, 0, 3, True)
                    for tt in range(3) for hf in range(2)],
                8: [lambda tt=tt, hf=hf: proj_pass(tt, hf, 0, 3, True)
                    for tt in range(3, 6) for hf in range(2)],
                9: [lambda tt=tt, hf=hf: proj_pass(tt, hf, 0, 3, True)
                    for tt in range(6, 8) for hf in range(2)],
                10: [lambda tt=tt, hf=hf: proj_pass(tt, hf, 3, 5, False)
                     for tt in range(6) for hf in range(2)],
                11: [lambda tt=tt, hf=hf: proj_pass(tt, hf, 3, 5, False)
                     for tt in range(6, 8) for hf in range(2)],
            }
            for h in range(12):
                head_attn(h, fills_by_head.get(h, []))

            # ============ projection pass 3 (chunk 5) + final add ============
            # the zpart partial rides into PSUM via an identity matmul and the
            # evacuation runs on ACT (idle once the exp stream drains), so the
            # tail isn't serialized on DVE adds
            for tt in range(8):
                accz = ps.tile([128, C], f32, tag="big")
                for c0, nc_ in ((0, 512), (512, 256)):
                    nc.tensor.matmul(
                        accz[:, c0:c0 + nc_],
                        yall[:, 5, tt * 128:(tt + 1) * 128],
                        wpr[:, 5, c0:c0 + nc_],
                        start=True, stop=False,
                    )
                    nc.tensor.matmul(
                        accz[:, c0:c0 + nc_],
                        iden[:],
                        zpart[:, tt, c0:c0 + nc_],
                        start=False, stop=True,
                    )
                z_t = sb.tile([128, C], f32, tag="Zt")
                nc.scalar.copy(z_t[:], accz[:])
                nc.sync.dma_start(out_d[tt * 128:(tt + 1) * 128, :], z_t[:])

    nc.compile()
    _cache["nc"] = nc
    return nc


def _host_prep(x, w_qkv, b_qkv, w_proj, b_proj, rel_pos_h, rel_pos_w):
    scale = HD ** -0.5
    w_qkv = _f32(w_qkv)
    b_qkv = _f32(b_qkv)

    w_qk = w_qkv[:, : 2 * C].copy()
    w_qk[:, :C] *= scale
    b_qk_flat = b_qkv[: 2 * C].copy()
    b_qk_flat[:C] *= scale
    b_qk = np.ascontiguousarray(b_qk_flat.reshape(12, 128).T)  # [128, 12]

    # relt [64, 2048]: cols tbl*1024 + qx*32 + j -> 8*rel_pos[qx - j + 31, :]
    idx = np.arange(32)[:, None] - np.arange(32)[None, :] + 31  # [qx, j]
    relt = np.concatenate(
        [
            (8.0 * _f32(rel_pos_h))[idx].transpose(2, 0, 1).reshape(64, 1024),
            (8.0 * _f32(rel_pos_w))[idx].transpose(2, 0, 1).reshape(64, 1024),
        ],
        axis=1,
    )

    k = np.arange(T)
    onehot = np.zeros((64, T), np.float32)
    onehot[k // 32, k] = 1.0
    onehot[32 + k % 32, k] = 1.0

    shared = {
        "w_qk": _bf(w_qk),
        "w_v": _bf(w_qkv[:, 2 * C:]),
        "w_p": _bf(w_proj),
        "b_qk": _f32(b_qk),
        "b_v": _f32(b_qkv[2 * C:])[None, :],
        "b_p": _f32(b_proj)[None, :],
        "relt": _bf(relt),
        "onehot": _bf(onehot),
        "ident": _bf(np.eye(128, dtype=np.float32)),
        "ones64": _bf(np.ones((1, 64), dtype=np.float32)),
    }
    x = _f32(x)
    in_maps = []
    for i in range(N_CORES):
        m = dict(shared)
        m["xT"] = _bf(x[i].reshape(T, C).T)
        in_maps.append(m)
    return in_maps


def kernel(x, w_qkv, b_qkv, w_proj, b_proj, rel_pos_h, rel_pos_w):
    from concourse.bass_utils import run_bass_kernel_spmd

    nc = _build_nc()
    in_maps = _host_prep(x, w_qkv, b_qkv, w_proj, b_proj, rel_pos_h, rel_pos_w)
    res = run_bass_kernel_spmd(nc, in_maps, core_ids=list(range(N_CORES)))
    out = np.stack([_f32(res.results[i]["out"]) for i in range(N_CORES)])
    return out.reshape(B, H, W, C)
